# revision 37
# baseline (speedup 1.0000x reference)
"""KNN cluster kernel for Trainium2 (8 NeuronCores, one batch per core).

Computes, for each of N=8 batches independently: squared L2 distances between
queries coords2[:, n, :] (L2=4096) and references coords1[:, n, :] (L1=4096)
in C=64 dims, then the indices of the 16 nearest references per query
(ascending distance). Output matches torch_cluster.knn-style flattened
(clusters, batch_idx) of the jax reference.

Device strategy per core:
  - Augmented transposed operands (K=66): rows 0..63 = Q^T / 2*X^T plus rows
    encoding (-q2)/(-x2), so one matmul yields s = 2*Q.X - q2 - x2 = -dist^2.
    Matmuls run in plain fp32 (f32r measured +4e-3 abs error on d^2 ->
    scrambles ~5k rankings; gap at the rank-16 boundary has median 0.23, so
    only sub-1e-4 perturbations are safe).
  - x-side operand prep runs once up front; q-side prep (norm, augment,
    transpose) is interleaved per-tile into the main loop so it overlaps
    with earlier tiles' compute.
  - Top-8 per 512-chunk on the DVE (max8 + find-index8 per chunk): 64
    candidate (value, chunk-local index) pairs per query. The DVE's two
    4096-element scans per query row are the bottleneck (no perf modes
    exist for max8/find-index8); everything else hides under them.
  - Host merges top-16-of-64 exactly in fp32 with jax top_k tie-breaking
    (value desc, then index asc) via a monotone composite integer key.
"""

import sys

import numpy as np

sys.path.insert(0, "/opt/trn_rl_repo")

L = 4096  # L1 == L2
N = 8
C = 64
K = 16
P = 128  # partitions / queries per tile
NT = L // P  # 32 query tiles
XC = 8  # matmul moving chunks of 512
MM_N = L // XC  # 512
NCH = 8  # top-k chunking of the 4096-wide row
CHW = L // NCH  # 512
NCAND = NCH * 8  # 64 candidates per query
KAUG = C + 2  # 66: contraction with -q2 / -x2 rows folded in

_CACHE = {}


def build_body(tc, q_ap, x_ap, ci_ap, cv_ap):
    from concourse import mybir, masks

    nc = tc.nc
    f32 = mybir.dt.float32
    u16 = mybir.dt.uint16

    with (
        tc.tile_pool(name="const", bufs=1) as const_pool,
        tc.tile_pool(name="inp", bufs=1) as inp_pool,
        tc.tile_pool(name="aug", bufs=1) as aug_pool,
        tc.tile_pool(name="tpsum", bufs=2, space="PSUM") as tpsum_pool,
        tc.tile_pool(name="npsum", bufs=2, space="PSUM") as npsum_pool,
        tc.tile_pool(name="mpsum", bufs=2, space="PSUM") as mpsum_pool,
        tc.tile_pool(name="s", bufs=3) as s_pool,
        tc.tile_pool(name="small", bufs=3) as small_pool,
    ):
        ident = const_pool.tile([P, P], f32)
        masks.make_identity(nc, ident[:])

        q_sb = inp_pool.tile([P, NT * C], f32)
        x_sb = inp_pool.tile([P, NT * C], f32)

        # Whole-tensor loads on the two parallel HWDGE queues (x via SP,
        # q via ACT) — one big DMA is ~5x cheaper in engine-busy than 32
        # per-tile DMAs, and the queues overlap.
        XG0 = 8
        nc.sync.dma_start(
            x_sb[:, 0 : XG0 * C].rearrange("p (t c) -> p t c", c=C),
            x_ap[0 : XG0 * P, :].rearrange("(t p) c -> p t c", p=P),
        )
        nc.scalar.dma_start(q_sb[:, 0:C], q_ap[0:P, :])
        nc.sync.dma_start(
            x_sb[:, XG0 * C :].rearrange("p (t c) -> p t c", c=C),
            x_ap[XG0 * P :, :].rearrange("(t p) c -> p t c", p=P),
        )
        nc.scalar.dma_start(
            q_sb[:, C:].rearrange("p (t c) -> p t c", c=C),
            q_ap[P:, :].rearrange("(t p) c -> p t c", p=P),
        )

        qT = aug_pool.tile([KAUG, L], f32)
        xT = aug_pool.tile([KAUG, L], f32)

        # Operand row layout (contraction K=66):
        #   qT rows: [2Q^T (0..63) | ones (64) | -q2 (65)]
        #   xT rows: [ X^T (0..63) | -x2 (64)  | ones (65)]
        # Norm + ones rows come from the PE: the squares staging tiles carry
        # a constant ones row at partition 64, and a [65, 2] weight picks
        # out [ones; -norm] as one [2, N] matmul output, so rows 64:66 are
        # written together (SBUF partition starts must be 0/32/64/96 — a
        # single-row write at partition 65 is illegal).
        XG = 8  # x-tiles per prep group
        wq = const_pool.tile([C + 1, 2], f32)  # cols: [ones-pick | -0.25*sum]
        wx = const_pool.tile([C + 1, 2], f32)  # cols: [-1*sum | ones-pick]
        nc.vector.memset(wq[0:C, 0:1], 0.0)
        nc.vector.memset(wq[0:C, 1:2], -0.25)
        nc.vector.memset(wx[0:C, 0:1], -1.0)
        nc.vector.memset(wx[0:C, 1:2], 0.0)
        nc.vector.memset(wq[C : C + 1, 0:1], 1.0)
        nc.vector.memset(wq[C : C + 1, 1:2], 0.0)
        nc.vector.memset(wx[C : C + 1, 0:1], 0.0)
        nc.vector.memset(wx[C : C + 1, 1:2], 1.0)
        sqq = aug_pool.tile([C + 1, P], f32, tag="sqq")
        nc.vector.memset(sqq[C : C + 1, :], 1.0)
        sqx_bufs = []
        for b in range(2):
            t_ = aug_pool.tile([C + 1, XG * P], f32, tag=f"sqx{b}")
            nc.vector.memset(t_[C : C + 1, :], 1.0)
            sqx_bufs.append(t_)

        def emit_xgroup(g):
            t0g, t1g = g * XG, (g + 1) * XG
            # coordinate transposes straight from x_sb, four tiles per PSUM
            # bank -> one wide scalar copy each (fixed copy cost amortized)
            for tp in range(t0g, t1g, 4):
                px = tpsum_pool.tile([KAUG, 4 * P], f32, tag="tps")
                for u in range(4):
                    nc.tensor.transpose(
                        px[0:C, u * P : (u + 1) * P],
                        x_sb[:, (tp + u) * C : (tp + u + 1) * C],
                        ident[:],
                    )
                nc.scalar.copy(xT[0:C, tp * P : (tp + 4) * P], px[0:C, :])
            # rows 64:66 = [-x2 ; ones]: square the group's X^T rows (the
            # staging tile carries a ones row), then wx^T @ sq -> [2, N]
            sqx = sqx_bufs[g % 2]
            nc.scalar.activation(
                sqx[0:C, :], xT[0:C, t0g * P : t1g * P],
                mybir.ActivationFunctionType.Square,
            )
            for h in range(2):
                pn = npsum_pool.tile([2, MM_N], f32, tag="norm")
                nc.tensor.matmul(
                    pn[:],
                    lhsT=wx[:],
                    rhs=sqx[:, h * MM_N : (h + 1) * MM_N],
                    start=True,
                    stop=True,
                )
                nc.scalar.copy(
                    xT[C : C + 2, t0g * P + h * MM_N : t0g * P + (h + 1) * MM_N],
                    pn[:],
                )

        def emit_qprep(t):
            # 2Q^T rows via transpose + x2 scale, then -q2 row via the same
            # square + ones-matmul trick (scale -0.25 since rows hold 2Q)
            pq = tpsum_pool.tile([KAUG, 4 * P], f32, tag="tps")
            nc.tensor.transpose(pq[0:C, 0:P], q_sb[:, t * C : (t + 1) * C], ident[:])
            nc.scalar.mul(qT[0:C, t * P : (t + 1) * P], pq[0:C, 0:P], 2.0)
            nc.scalar.activation(
                sqq[0:C, :], qT[0:C, t * P : (t + 1) * P],
                mybir.ActivationFunctionType.Square,
            )
            pn = npsum_pool.tile([2, MM_N], f32, tag="norm")
            nc.tensor.matmul(
                pn[0:2, 0:P], lhsT=wq[:], rhs=sqq[:], start=True, stop=True
            )
            nc.scalar.copy(qT[C : C + 2, t * P : (t + 1) * P], pn[0:2, 0:P])

        def emit_chunk(t, j2, s_sb, cand_v, ci_t):
            # Two matmul chunks share one 2-bank PSUM tile and one wide
            # scalar copy (activation PSUM reads are expensive on HW; this
            # halves their count), then the two DVE scans run per 512-chunk.
            ps = mpsum_pool.tile([P, 2 * MM_N], f32, tag="mm")
            for h in range(2):
                j = 2 * j2 + h
                nc.tensor.matmul(
                    ps[:, h * MM_N : (h + 1) * MM_N],
                    lhsT=qT[:, t * P : (t + 1) * P],
                    rhs=xT[:, j * MM_N : (j + 1) * MM_N],
                    start=True,
                    stop=True,
                )
            nc.scalar.copy(s_sb[:, 2 * j2 * MM_N : (2 * j2 + 2) * MM_N], ps[:])
            for h in range(2):
                j = 2 * j2 + h
                nc.vector.max(
                    cand_v[:, j * 8 : (j + 1) * 8],
                    s_sb[:, j * CHW : (j + 1) * CHW],
                )
                nc.vector.max_index(
                    ci_t[:, j * 8 : (j + 1) * 8],
                    cand_v[:, j * 8 : (j + 1) * 8],
                    s_sb[:, j * CHW : (j + 1) * CHW],
                )

        # ---- main loop ----
        # Tile 0's chunk j only needs x-group j//2, so the four x-prep groups
        # are emitted interleaved with tile 0's chunks: each engine stream
        # keeps producers ahead of consumers and the DVE starts ~30us sooner.
        for t in range(NT):
            if t == 0:
                emit_xgroup(0)
            emit_qprep(t)
            s_sb = s_pool.tile([P, L], f32, tag="s")
            cand_v = small_pool.tile([P, NCAND], f32, tag="cand_v")
            ci_t = small_pool.tile([P, NCAND], u16, tag="ci")
            for j2 in range(XC // 2):
                emit_chunk(t, j2, s_sb, cand_v, ci_t)
                if t == 0 and j2 < XC // 2 - 1:
                    emit_xgroup(j2 + 1)
            nc.sync.dma_start(ci_ap[t * P : (t + 1) * P, :], ci_t[:])
            nc.sync.dma_start(cv_ap[t * P : (t + 1) * P, :], cand_v[:])


def _build_program(reps=1):
    from concourse import bacc, mybir, tile

    nc = bacc.Bacc(
        "TRN2",
        target_bir_lowering=False,
        debug=False,
        enable_asserts=True,
        num_devices=N,
    )
    q_dram = nc.dram_tensor("q", [L, C], mybir.dt.float32, kind="ExternalInput")
    x_dram = nc.dram_tensor("x", [L, C], mybir.dt.float32, kind="ExternalInput")
    ci_dram = nc.dram_tensor("ci", [L, NCAND], mybir.dt.uint16, kind="ExternalOutput")
    cv_dram = nc.dram_tensor("cv", [L, NCAND], mybir.dt.float32, kind="ExternalOutput")

    with tile.TileContext(nc) as tc:
        for _ in range(reps):
            build_body(tc, q_dram.ap(), x_dram.ap(), ci_dram.ap(), cv_dram.ap())

    nc.compile()
    return nc


def _get_nc():
    if "nc" not in _CACHE:
        _CACHE["nc"] = _build_program()
    return _CACHE["nc"]


_CHUNK_BASE = (np.arange(NCAND, dtype=np.int64) >> 3) * CHW  # candidate -> chunk offset


def _postprocess(ci, cv):
    # ci: (L, 64) uint16 chunk-local indices; cv: (L, 64) f32 candidate values
    # (s = -d^2, per-chunk top-8, descending within each chunk's 8).
    gidx = ci.astype(np.uint64) + _CHUNK_BASE.astype(np.uint64)[None, :]
    # Exact top-16-of-64 with jax top_k tie rule (value desc, then index asc)
    # via a single monotone composite key: float32 bits -> order-isomorphic
    # uint32, complemented for descending, index in the low 12 bits.
    b = np.ascontiguousarray(cv).view(np.uint32).astype(np.uint64)
    k_asc = np.where(b >> 31, ~b & 0xFFFFFFFF, b | 0x80000000)
    comp = ((k_asc ^ 0xFFFFFFFF) << 12) | gidx
    top = np.sort(np.partition(comp, K - 1, axis=1)[:, :K], axis=1)
    return (top & 0xFFF).astype(np.int64)  # (L, K) global candidate indices


def _in_maps(inputs):
    coords1 = np.asarray(inputs["coords1"])
    coords2 = np.asarray(inputs["coords2"])
    return [
        {
            "q": np.ascontiguousarray(coords2[:, n, :], dtype=np.float32),
            "x": np.ascontiguousarray(coords1[:, n, :], dtype=np.float32),
        }
        for n in range(N)
    ]


def kernel(coords1, coords2, k):
    from concourse.bass_utils import run_bass_kernel_spmd

    coords1 = np.asarray(coords1)
    coords2 = np.asarray(coords2)
    assert int(k) == K, f"kernel hardcoded for k={K}, got {k}"
    assert coords1.shape == (L, N, C) and coords2.shape == (L, N, C)

    nc = _get_nc()
    in_maps = _in_maps({"coords1": coords1, "coords2": coords2})
    res = run_bass_kernel_spmd(nc, in_maps, core_ids=list(range(N)))
    local = np.stack(
        [_postprocess(r["ci"], r["cv"]) for r in res.results], axis=0
    )  # (N, L, K)
    # global_idx = local + n*L1 ; clusters = global_idx mod L2 == local (L1==L2)
    clusters = np.transpose(local, (2, 1, 0)).astype(np.int32).reshape(-1)
    batch_idx = np.broadcast_to(
        np.arange(N, dtype=np.int32), (K, L, N)
    ).reshape(-1)
    return clusters, batch_idx


# revision 39
# speedup vs baseline: 1.0791x; 1.0791x over previous
"""KNN cluster kernel for Trainium2 (8 NeuronCores, one batch per core).

Computes, for each of N=8 batches independently: squared L2 distances between
queries coords2[:, n, :] (L2=4096) and references coords1[:, n, :] (L1=4096)
in C=64 dims, then the indices of the 16 nearest references per query
(ascending distance). Output matches torch_cluster.knn-style flattened
(clusters, batch_idx) of the jax reference.

Device strategy per core:
  - Augmented transposed operands (K=66): rows [2Q^T | ones | -q2] against
    [X^T | -x2 | ones], so one fp32 matmul yields s = 2*Q.X - q2 - x2 =
    -dist^2 in PSUM. Plain fp32 on purpose: f32r measured +4e-3 abs error
    on d^2, scrambling ~5k rankings (the gap at the rank-16 boundary has
    median 0.23, so only sub-1e-4 perturbations are safe).
  - Norm rows are computed on the PE too: the squares staging carries a
    constant ones row, and a [65, 2] weight produces [ones; -norm] as one
    [2, N] output (SBUF partition starts must be 0/32/64/96, so rows 64:66
    are written together).
  - x-side prep (transposes 4-per-PSUM-bank + wide copies) is emitted in 4
    groups interleaved with tile 0's chunks; q-side prep rides inside each
    tile iteration. Inputs stream in via both HWDGE queues.
  - Top-8 per 512-chunk on the DVE (max8 + find-index8 right after each
    chunk's matmul): 64 candidate (value, chunk-local index) pairs per
    query. The DVE's two 4096-element scans per query row are the hard
    bottleneck (max8/find-index8 have no perf modes at any dtype; ~91%
    DVE occupancy in the cost-model timeline); everything else hides
    under them.
  - Host merges top-16-of-64 exactly in fp32 with jax top_k tie-breaking
    (value desc, then index asc) via a monotone composite integer key.
"""

import sys

import numpy as np

sys.path.insert(0, "/opt/trn_rl_repo")

L = 4096  # L1 == L2
N = 8
C = 64
K = 16
P = 128  # partitions / queries per tile
NT = L // P  # 32 query tiles
XC = 8  # matmul moving chunks of 512
MM_N = L // XC  # 512
NCH = 8  # top-k chunking of the 4096-wide row
CHW = L // NCH  # 512
NCAND = NCH * 8  # 64 candidates per query
KAUG = C + 2  # 66: contraction with -q2 / -x2 rows folded in

_CACHE = {}


def build_body(tc, q_ap, x_ap, ci_ap, cv_ap):
    from concourse import mybir, masks

    nc = tc.nc
    f32 = mybir.dt.float32
    u16 = mybir.dt.uint16

    with (
        tc.tile_pool(name="const", bufs=1) as const_pool,
        tc.tile_pool(name="inp", bufs=1) as inp_pool,
        tc.tile_pool(name="aug", bufs=1) as aug_pool,
        tc.tile_pool(name="tpsum", bufs=2, space="PSUM") as tpsum_pool,
        tc.tile_pool(name="npsum", bufs=2, space="PSUM") as npsum_pool,
        tc.tile_pool(name="mpsum", bufs=4, space="PSUM") as mpsum_pool,
        tc.tile_pool(name="s", bufs=3) as s_pool,
        tc.tile_pool(name="small", bufs=3) as small_pool,
    ):
        ident = const_pool.tile([P, P], f32)
        masks.make_identity(nc, ident[:])

        q_sb = inp_pool.tile([P, NT * C], f32)
        x_sb = inp_pool.tile([P, NT * C], f32)

        # Whole-tensor loads on the two parallel HWDGE queues (x via SP,
        # q via ACT) — one big DMA is ~5x cheaper in engine-busy than 32
        # per-tile DMAs, and the queues overlap.
        XG0 = 8
        nc.sync.dma_start(
            x_sb[:, 0 : XG0 * C].rearrange("p (t c) -> p t c", c=C),
            x_ap[0 : XG0 * P, :].rearrange("(t p) c -> p t c", p=P),
        )
        nc.scalar.dma_start(q_sb[:, 0:C], q_ap[0:P, :])
        nc.sync.dma_start(
            x_sb[:, XG0 * C :].rearrange("p (t c) -> p t c", c=C),
            x_ap[XG0 * P :, :].rearrange("(t p) c -> p t c", p=P),
        )
        nc.scalar.dma_start(
            q_sb[:, C:].rearrange("p (t c) -> p t c", c=C),
            q_ap[P:, :].rearrange("(t p) c -> p t c", p=P),
        )

        qT = aug_pool.tile([KAUG, L], f32)
        xT = aug_pool.tile([KAUG, L], f32)

        # Operand row layout (contraction K=66):
        #   qT rows: [2Q^T (0..63) | ones (64) | -q2 (65)]
        #   xT rows: [ X^T (0..63) | -x2 (64)  | ones (65)]
        # Norm + ones rows come from the PE: the squares staging tiles carry
        # a constant ones row at partition 64, and a [65, 2] weight picks
        # out [ones; -norm] as one [2, N] matmul output, so rows 64:66 are
        # written together (SBUF partition starts must be 0/32/64/96 — a
        # single-row write at partition 65 is illegal).
        XG = 8  # x-tiles per prep group
        wq = const_pool.tile([C + 1, 2], f32)  # cols: [ones-pick | -0.25*sum]
        wx = const_pool.tile([C + 1, 2], f32)  # cols: [-1*sum | ones-pick]
        nc.vector.memset(wq[0:C, 0:1], 0.0)
        nc.vector.memset(wq[0:C, 1:2], -0.25)
        nc.vector.memset(wx[0:C, 0:1], -1.0)
        nc.vector.memset(wx[0:C, 1:2], 0.0)
        nc.vector.memset(wq[C : C + 1, 0:1], 1.0)
        nc.vector.memset(wq[C : C + 1, 1:2], 0.0)
        nc.vector.memset(wx[C : C + 1, 0:1], 0.0)
        nc.vector.memset(wx[C : C + 1, 1:2], 1.0)
        sqq = aug_pool.tile([C + 1, P], f32, tag="sqq")
        nc.vector.memset(sqq[C : C + 1, :], 1.0)
        sqx_bufs = []
        for b in range(2):
            t_ = aug_pool.tile([C + 1, XG * P], f32, tag=f"sqx{b}")
            nc.vector.memset(t_[C : C + 1, :], 1.0)
            sqx_bufs.append(t_)

        def emit_xgroup(g):
            t0g, t1g = g * XG, (g + 1) * XG
            # coordinate transposes straight from x_sb, four tiles per PSUM
            # bank -> one wide scalar copy each (fixed copy cost amortized)
            for tp in range(t0g, t1g, 4):
                px = tpsum_pool.tile([KAUG, 4 * P], f32, tag="tps")
                for u in range(4):
                    nc.tensor.transpose(
                        px[0:C, u * P : (u + 1) * P],
                        x_sb[:, (tp + u) * C : (tp + u + 1) * C],
                        ident[:],
                    )
                nc.scalar.copy(xT[0:C, tp * P : (tp + 4) * P], px[0:C, :])
            # rows 64:66 = [-x2 ; ones]: square the group's X^T rows (the
            # staging tile carries a ones row), then wx^T @ sq -> [2, N]
            sqx = sqx_bufs[g % 2]
            nc.scalar.activation(
                sqx[0:C, :], xT[0:C, t0g * P : t1g * P],
                mybir.ActivationFunctionType.Square,
            )
            for h in range(2):
                pn = npsum_pool.tile([2, MM_N], f32, tag="norm")
                nc.tensor.matmul(
                    pn[:],
                    lhsT=wx[:],
                    rhs=sqx[:, h * MM_N : (h + 1) * MM_N],
                    start=True,
                    stop=True,
                )
                nc.scalar.copy(
                    xT[C : C + 2, t0g * P + h * MM_N : t0g * P + (h + 1) * MM_N],
                    pn[:],
                )

        def emit_qprep(t):
            # 2Q^T rows via transpose + x2 scale, then -q2 row via the same
            # square + ones-matmul trick (scale -0.25 since rows hold 2Q)
            pq = tpsum_pool.tile([KAUG, 4 * P], f32, tag="tps")
            nc.tensor.transpose(pq[0:C, 0:P], q_sb[:, t * C : (t + 1) * C], ident[:])
            nc.scalar.mul(qT[0:C, t * P : (t + 1) * P], pq[0:C, 0:P], 2.0)
            nc.scalar.activation(
                sqq[0:C, :], qT[0:C, t * P : (t + 1) * P],
                mybir.ActivationFunctionType.Square,
            )
            pn = npsum_pool.tile([2, MM_N], f32, tag="norm")
            nc.tensor.matmul(
                pn[0:2, 0:P], lhsT=wq[:], rhs=sqq[:], start=True, stop=True
            )
            nc.scalar.copy(qT[C : C + 2, t * P : (t + 1) * P], pn[0:2, 0:P])

        def emit_chunk(t, j, s_sb, cand_v, ci_t):
            ps = mpsum_pool.tile([P, MM_N], f32, tag="mm")
            nc.tensor.matmul(
                ps[:],
                lhsT=qT[:, t * P : (t + 1) * P],
                rhs=xT[:, j * MM_N : (j + 1) * MM_N],
                start=True,
                stop=True,
            )
            nc.scalar.copy(s_sb[:, j * MM_N : (j + 1) * MM_N], ps[:])
            nc.vector.max(
                cand_v[:, j * 8 : (j + 1) * 8],
                s_sb[:, j * CHW : (j + 1) * CHW],
            )
            nc.vector.max_index(
                ci_t[:, j * 8 : (j + 1) * 8],
                cand_v[:, j * 8 : (j + 1) * 8],
                s_sb[:, j * CHW : (j + 1) * CHW],
            )

        # ---- main loop ----
        # Tile 0's chunk j only needs x-group j//2, so the four x-prep groups
        # are emitted interleaved with tile 0's chunks: each engine stream
        # keeps producers ahead of consumers and the DVE starts ~30us sooner.
        for t in range(NT):
            if t == 0:
                emit_xgroup(0)
            emit_qprep(t)
            s_sb = s_pool.tile([P, L], f32, tag="s")
            cand_v = small_pool.tile([P, NCAND], f32, tag="cand_v")
            ci_t = small_pool.tile([P, NCAND], u16, tag="ci")
            for j in range(XC):
                emit_chunk(t, j, s_sb, cand_v, ci_t)
                if t == 0 and j % 2 == 1 and j < XC - 1:
                    emit_xgroup(j // 2 + 1)
            nc.sync.dma_start(ci_ap[t * P : (t + 1) * P, :], ci_t[:])
            nc.sync.dma_start(cv_ap[t * P : (t + 1) * P, :], cand_v[:])


def _build_program(reps=1):
    from concourse import bacc, mybir, tile

    nc = bacc.Bacc(
        "TRN2",
        target_bir_lowering=False,
        debug=False,
        enable_asserts=True,
        num_devices=N,
    )
    q_dram = nc.dram_tensor("q", [L, C], mybir.dt.float32, kind="ExternalInput")
    x_dram = nc.dram_tensor("x", [L, C], mybir.dt.float32, kind="ExternalInput")
    ci_dram = nc.dram_tensor("ci", [L, NCAND], mybir.dt.uint16, kind="ExternalOutput")
    cv_dram = nc.dram_tensor("cv", [L, NCAND], mybir.dt.float32, kind="ExternalOutput")

    with tile.TileContext(nc) as tc:
        for _ in range(reps):
            build_body(tc, q_dram.ap(), x_dram.ap(), ci_dram.ap(), cv_dram.ap())

    nc.compile()
    return nc


def _get_nc():
    if "nc" not in _CACHE:
        _CACHE["nc"] = _build_program()
    return _CACHE["nc"]


_CHUNK_BASE = (np.arange(NCAND, dtype=np.int64) >> 3) * CHW  # candidate -> chunk offset


def _postprocess(ci, cv):
    # ci: (L, 64) uint16 chunk-local indices; cv: (L, 64) f32 candidate values
    # (s = -d^2, per-chunk top-8, descending within each chunk's 8).
    gidx = ci.astype(np.uint64) + _CHUNK_BASE.astype(np.uint64)[None, :]
    # Exact top-16-of-64 with jax top_k tie rule (value desc, then index asc)
    # via a single monotone composite key: float32 bits -> order-isomorphic
    # uint32, complemented for descending, index in the low 12 bits.
    b = np.ascontiguousarray(cv).view(np.uint32).astype(np.uint64)
    k_asc = np.where(b >> 31, ~b & 0xFFFFFFFF, b | 0x80000000)
    comp = ((k_asc ^ 0xFFFFFFFF) << 12) | gidx
    top = np.sort(np.partition(comp, K - 1, axis=1)[:, :K], axis=1)
    return (top & 0xFFF).astype(np.int64)  # (L, K) global candidate indices


def _in_maps(inputs):
    coords1 = np.asarray(inputs["coords1"])
    coords2 = np.asarray(inputs["coords2"])
    return [
        {
            "q": np.ascontiguousarray(coords2[:, n, :], dtype=np.float32),
            "x": np.ascontiguousarray(coords1[:, n, :], dtype=np.float32),
        }
        for n in range(N)
    ]


def kernel(coords1, coords2, k):
    from concourse.bass_utils import run_bass_kernel_spmd

    coords1 = np.asarray(coords1)
    coords2 = np.asarray(coords2)
    assert int(k) == K, f"kernel hardcoded for k={K}, got {k}"
    assert coords1.shape == (L, N, C) and coords2.shape == (L, N, C)

    nc = _get_nc()
    in_maps = _in_maps({"coords1": coords1, "coords2": coords2})
    res = run_bass_kernel_spmd(nc, in_maps, core_ids=list(range(N)))
    local = np.stack(
        [_postprocess(r["ci"], r["cv"]) for r in res.results], axis=0
    )  # (N, L, K)
    # global_idx = local + n*L1 ; clusters = global_idx mod L2 == local (L1==L2)
    clusters = np.transpose(local, (2, 1, 0)).astype(np.int32).reshape(-1)
    batch_idx = np.broadcast_to(
        np.arange(N, dtype=np.int32), (K, L, N)
    ).reshape(-1)
    return clusters, batch_idx


# revision 40
# speedup vs baseline: 1.2544x; 1.1625x over previous
"""KNN cluster kernel for Trainium2 (8 NeuronCores, one batch per core).

Computes, for each of N=8 batches independently: squared L2 distances between
queries coords2[:, n, :] (L2=4096) and references coords1[:, n, :] (L1=4096)
in C=64 dims, then the indices of the 16 nearest references per query
(ascending distance). Output matches torch_cluster.knn-style flattened
(clusters, batch_idx) of the jax reference.

Device strategy per core:
  - Augmented transposed operands (K=66): rows [2Q^T | ones | -q2] against
    [X^T | -x2 | ones], so one fp32 matmul yields s = 2*Q.X - q2 - x2 =
    -dist^2 in PSUM. Plain fp32 on purpose: f32r measured +4e-3 abs error
    on d^2, scrambling ~5k rankings (the gap at the rank-16 boundary has
    median 0.23, so only sub-1e-4 perturbations are safe).
  - Norm rows are computed on the PE too: the squares staging carries a
    constant ones row, and a [65, 2] weight produces [ones; -norm] as one
    [2, N] output (SBUF partition starts must be 0/32/64/96, so rows 64:66
    are written together).
  - x-side prep (transposes 4-per-PSUM-bank + wide copies) is emitted in 4
    groups interleaved with tile 0's chunks; q-side prep rides inside each
    tile iteration. Inputs stream in via both HWDGE queues.
  - Top-8 per 512-chunk on the DVE (max8 + find-index8 right after each
    chunk's matmul): 64 candidate (value, chunk-local index) pairs per
    query. The DVE's two 4096-element scans per query row are the hard
    bottleneck (max8/find-index8 have no perf modes at any dtype; ~91%
    DVE occupancy in the cost-model timeline); everything else hides
    under them.
  - Host merges top-16-of-64 exactly in fp32 with jax top_k tie-breaking
    (value desc, then index asc) via a monotone composite integer key.
"""

import sys

import numpy as np

sys.path.insert(0, "/opt/trn_rl_repo")

L = 4096  # L1 == L2
N = 8
C = 64
K = 16
P = 128  # partitions / queries per tile
NT = L // P  # 32 query tiles
XC = 8  # matmul moving chunks of 512
MM_N = L // XC  # 512
NCH = 8  # top-k chunking of the 4096-wide row
CHW = L // NCH  # 512
NCAND = NCH * 8  # 64 candidates per query
KAUG = C + 2  # 66: contraction with -q2 / -x2 rows folded in

_CACHE = {}


def build_body(tc, q_ap, x_ap, ci_ap, cv_ap):
    from concourse import mybir, masks

    nc = tc.nc
    f32 = mybir.dt.float32
    u16 = mybir.dt.uint16

    with (
        tc.tile_pool(name="const", bufs=1) as const_pool,
        tc.tile_pool(name="inp", bufs=1) as inp_pool,
        tc.tile_pool(name="aug", bufs=1) as aug_pool,
        tc.tile_pool(name="tpsum", bufs=2, space="PSUM") as tpsum_pool,
        tc.tile_pool(name="npsum", bufs=2, space="PSUM") as npsum_pool,
        tc.tile_pool(name="mpsum", bufs=4, space="PSUM") as mpsum_pool,
        tc.tile_pool(name="s", bufs=3) as s_pool,
        tc.tile_pool(name="small", bufs=3) as small_pool,
    ):
        ident = const_pool.tile([P, P], f32)
        masks.make_identity(nc, ident[:])

        q_sb = inp_pool.tile([P, NT * C], f32)
        x_sb = inp_pool.tile([P, NT * C], f32)

        # Whole-tensor loads on the two parallel HWDGE queues (x via SP,
        # q via ACT) — one big DMA is ~5x cheaper in engine-busy than 32
        # per-tile DMAs, and the queues overlap.
        XG0 = 8
        nc.sync.dma_start(
            x_sb[:, 0 : XG0 * C].rearrange("p (t c) -> p t c", c=C),
            x_ap[0 : XG0 * P, :].rearrange("(t p) c -> p t c", p=P),
        )
        nc.scalar.dma_start(q_sb[:, 0:C], q_ap[0:P, :])
        nc.sync.dma_start(
            x_sb[:, XG0 * C :].rearrange("p (t c) -> p t c", c=C),
            x_ap[XG0 * P :, :].rearrange("(t p) c -> p t c", p=P),
        )
        nc.scalar.dma_start(
            q_sb[:, C:].rearrange("p (t c) -> p t c", c=C),
            q_ap[P:, :].rearrange("(t p) c -> p t c", p=P),
        )

        qT = aug_pool.tile([KAUG, L], f32)
        xT = aug_pool.tile([KAUG, L], f32)

        # Operand row layout (contraction K=66):
        #   qT rows: [2Q^T (0..63) | ones (64) | -q2 (65)]
        #   xT rows: [ X^T (0..63) | -x2 (64)  | ones (65)]
        # Norm + ones rows come from the PE: the squares staging tiles carry
        # a constant ones row at partition 64, and a [65, 2] weight picks
        # out [ones; -norm] as one [2, N] matmul output, so rows 64:66 are
        # written together (SBUF partition starts must be 0/32/64/96 — a
        # single-row write at partition 65 is illegal).
        XG = 8  # x-tiles per prep group
        wq = const_pool.tile([C + 1, 2], f32)  # cols: [ones-pick | -0.25*sum]
        wx = const_pool.tile([C + 1, 2], f32)  # cols: [-1*sum | ones-pick]
        nc.vector.memset(wq[0:C, 0:1], 0.0)
        nc.vector.memset(wq[0:C, 1:2], -0.25)
        nc.vector.memset(wx[0:C, 0:1], -1.0)
        nc.vector.memset(wx[0:C, 1:2], 0.0)
        nc.vector.memset(wq[C : C + 1, 0:1], 1.0)
        nc.vector.memset(wq[C : C + 1, 1:2], 0.0)
        nc.vector.memset(wx[C : C + 1, 0:1], 0.0)
        nc.vector.memset(wx[C : C + 1, 1:2], 1.0)
        sqq = aug_pool.tile([C + 1, P], f32, tag="sqq")
        nc.vector.memset(sqq[C : C + 1, :], 1.0)
        sqx_bufs = []
        for b in range(2):
            t_ = aug_pool.tile([C + 1, XG * P], f32, tag=f"sqx{b}")
            nc.vector.memset(t_[C : C + 1, :], 1.0)
            sqx_bufs.append(t_)

        def emit_xgroup(g):
            t0g, t1g = g * XG, (g + 1) * XG
            # coordinate transposes straight from x_sb, four tiles per PSUM
            # bank -> one wide scalar copy each (fixed copy cost amortized)
            for tp in range(t0g, t1g, 4):
                px = tpsum_pool.tile([KAUG, 4 * P], f32, tag="tps")
                for u in range(4):
                    nc.tensor.transpose(
                        px[0:C, u * P : (u + 1) * P],
                        x_sb[:, (tp + u) * C : (tp + u + 1) * C],
                        ident[:],
                    )
                nc.scalar.copy(xT[0:C, tp * P : (tp + 4) * P], px[0:C, :])
            # rows 64:66 = [-x2 ; ones]: square the group's X^T rows (the
            # staging tile carries a ones row), then wx^T @ sq -> [2, N]
            sqx = sqx_bufs[g % 2]
            nc.scalar.activation(
                sqx[0:C, :], xT[0:C, t0g * P : t1g * P],
                mybir.ActivationFunctionType.Square,
            )
            for h in range(2):
                pn = npsum_pool.tile([2, MM_N], f32, tag="norm")
                nc.tensor.matmul(
                    pn[:],
                    lhsT=wx[:],
                    rhs=sqx[:, h * MM_N : (h + 1) * MM_N],
                    start=True,
                    stop=True,
                )
                nc.scalar.copy(
                    xT[C : C + 2, t0g * P + h * MM_N : t0g * P + (h + 1) * MM_N],
                    pn[:],
                )

        def emit_qprep(t):
            # 2Q^T rows via transpose + x2 scale, then -q2 row via the same
            # square + ones-matmul trick (scale -0.25 since rows hold 2Q)
            pq = tpsum_pool.tile([KAUG, 4 * P], f32, tag="tps")
            nc.tensor.transpose(pq[0:C, 0:P], q_sb[:, t * C : (t + 1) * C], ident[:])
            nc.scalar.mul(qT[0:C, t * P : (t + 1) * P], pq[0:C, 0:P], 2.0)
            nc.scalar.activation(
                sqq[0:C, :], qT[0:C, t * P : (t + 1) * P],
                mybir.ActivationFunctionType.Square,
            )
            pn = npsum_pool.tile([2, MM_N], f32, tag="norm")
            nc.tensor.matmul(
                pn[0:2, 0:P], lhsT=wq[:], rhs=sqq[:], start=True, stop=True
            )
            nc.scalar.copy(qT[C : C + 2, t * P : (t + 1) * P], pn[0:2, 0:P])

        def emit_chunk(t, j, s_sb, cand_v, ci_t):
            # DVE scans read the PSUM bank directly: no PSUM->SBUF copy, no
            # scalar engine in the chunk path, one PE->DVE dependency hop.
            ps = mpsum_pool.tile([P, MM_N], f32, tag="mm")
            nc.tensor.matmul(
                ps[:],
                lhsT=qT[:, t * P : (t + 1) * P],
                rhs=xT[:, j * MM_N : (j + 1) * MM_N],
                start=True,
                stop=True,
            )
            nc.vector.max(
                cand_v[:, j * 8 : (j + 1) * 8],
                ps[:],
            )
            nc.vector.max_index(
                ci_t[:, j * 8 : (j + 1) * 8],
                cand_v[:, j * 8 : (j + 1) * 8],
                ps[:],
            )

        # ---- main loop ----
        # Tile 0's chunk j only needs x-group j//2, so the four x-prep groups
        # are emitted interleaved with tile 0's chunks: each engine stream
        # keeps producers ahead of consumers and the DVE starts ~30us sooner.
        for t in range(NT):
            if t == 0:
                emit_xgroup(0)
            emit_qprep(t)
            s_sb = None
            cand_v = small_pool.tile([P, NCAND], f32, tag="cand_v")
            ci_t = small_pool.tile([P, NCAND], u16, tag="ci")
            for j in range(XC):
                emit_chunk(t, j, s_sb, cand_v, ci_t)
                if t == 0 and j % 2 == 1 and j < XC - 1:
                    emit_xgroup(j // 2 + 1)
            nc.sync.dma_start(ci_ap[t * P : (t + 1) * P, :], ci_t[:])
            nc.sync.dma_start(cv_ap[t * P : (t + 1) * P, :], cand_v[:])


def _build_program(reps=1):
    from concourse import bacc, mybir, tile

    nc = bacc.Bacc(
        "TRN2",
        target_bir_lowering=False,
        debug=False,
        enable_asserts=True,
        num_devices=N,
    )
    q_dram = nc.dram_tensor("q", [L, C], mybir.dt.float32, kind="ExternalInput")
    x_dram = nc.dram_tensor("x", [L, C], mybir.dt.float32, kind="ExternalInput")
    ci_dram = nc.dram_tensor("ci", [L, NCAND], mybir.dt.uint16, kind="ExternalOutput")
    cv_dram = nc.dram_tensor("cv", [L, NCAND], mybir.dt.float32, kind="ExternalOutput")

    with tile.TileContext(nc) as tc:
        for _ in range(reps):
            build_body(tc, q_dram.ap(), x_dram.ap(), ci_dram.ap(), cv_dram.ap())

    nc.compile()
    return nc


def _get_nc():
    if "nc" not in _CACHE:
        _CACHE["nc"] = _build_program()
    return _CACHE["nc"]


_CHUNK_BASE = (np.arange(NCAND, dtype=np.int64) >> 3) * CHW  # candidate -> chunk offset


def _postprocess(ci, cv):
    # ci: (L, 64) uint16 chunk-local indices; cv: (L, 64) f32 candidate values
    # (s = -d^2, per-chunk top-8, descending within each chunk's 8).
    gidx = ci.astype(np.uint64) + _CHUNK_BASE.astype(np.uint64)[None, :]
    # Exact top-16-of-64 with jax top_k tie rule (value desc, then index asc)
    # via a single monotone composite key: float32 bits -> order-isomorphic
    # uint32, complemented for descending, index in the low 12 bits.
    b = np.ascontiguousarray(cv).view(np.uint32).astype(np.uint64)
    k_asc = np.where(b >> 31, ~b & 0xFFFFFFFF, b | 0x80000000)
    comp = ((k_asc ^ 0xFFFFFFFF) << 12) | gidx
    top = np.sort(np.partition(comp, K - 1, axis=1)[:, :K], axis=1)
    return (top & 0xFFF).astype(np.int64)  # (L, K) global candidate indices


def _in_maps(inputs):
    coords1 = np.asarray(inputs["coords1"])
    coords2 = np.asarray(inputs["coords2"])
    return [
        {
            "q": np.ascontiguousarray(coords2[:, n, :], dtype=np.float32),
            "x": np.ascontiguousarray(coords1[:, n, :], dtype=np.float32),
        }
        for n in range(N)
    ]


def kernel(coords1, coords2, k):
    from concourse.bass_utils import run_bass_kernel_spmd

    coords1 = np.asarray(coords1)
    coords2 = np.asarray(coords2)
    assert int(k) == K, f"kernel hardcoded for k={K}, got {k}"
    assert coords1.shape == (L, N, C) and coords2.shape == (L, N, C)

    nc = _get_nc()
    in_maps = _in_maps({"coords1": coords1, "coords2": coords2})
    res = run_bass_kernel_spmd(nc, in_maps, core_ids=list(range(N)))
    local = np.stack(
        [_postprocess(r["ci"], r["cv"]) for r in res.results], axis=0
    )  # (N, L, K)
    # global_idx = local + n*L1 ; clusters = global_idx mod L2 == local (L1==L2)
    clusters = np.transpose(local, (2, 1, 0)).astype(np.int32).reshape(-1)
    batch_idx = np.broadcast_to(
        np.arange(N, dtype=np.int32), (K, L, N)
    ).reshape(-1)
    return clusters, batch_idx
